# revision 18
# baseline (speedup 1.0000x reference)
"""Trainium2 Bass kernel for nn_CumulantNN (mean-field spin dynamics).

Math (from the oracle):
    Jeff = 2*sigmoid(Js) - 1 = tanh(Js/2)            # constant [N, N]
    per Euler step t (64 steps):
        h  = Jeff @ sz                               # the dominant matvec
        pt = tanh(0.5 * (vs @ cos(2*pi*t*nb)))       # [N]
        sx += dt * (-2 h sy)
        sy += dt * ( 2 h sx - 2 pt sz)
        sz += dt * ( 2 pt sy)

Strategy (8 NeuronCores):
    - Row-shard Jeff: core c owns rows [c*1024, (c+1)*1024).
    - Keep Jeff^T SBUF-resident in bf16 ([128 j-partitions, 64 j-chunks x 1024
      rows] = 128 KB/partition). Setup: SWDGE cast-DMA fp32->bf16, PE-transpose
      128x128 blocks, fused tanh(0.5x) on ScalarE evacuating PSUM->SBUF.
    - Per step, the matvec runs on the TensorEngine: sz chunk [128,1] is the
      stationary operand, Jeff^T streams as the moving operand (N=512), 64
      accumulating matmuls per output half -> h [1, 1024] in PSUM.
    - h slices are AllGather'd (4 KB/core) through HBM bounce buffers; every
      core then applies the identical full-state update (state replicated,
      [64, 128] fp32 tiles).
    - Pipelining: sz(t+1) depends only on pt(t), sy(t) (NOT on h(t)), so the
      next step's stationary operand (PE-transposed sz) is ready mid-matvec
      and the TensorEngine never waits for the collective; the gather + sx/sy
      updates hide under the next 28 us matvec.

The dynamics blow up (|h|~40, dt=1/64 -> explicit Euler diverges
super-exponentially); the reference output is all-NaN and this kernel
faithfully reproduces that (bf16 vs fp32 makes no difference to the fate).
"""

import sys

import numpy as np

if "/opt/trn_rl_repo" not in sys.path:
    sys.path.insert(0, "/opt/trn_rl_repo")

N = 8192
NB = 5
NSTEPS = 64
NCORES = 8
ROWS = N // NCORES          # 1024 rows per core
P = 128                     # partitions
JCH = N // P                # 64 j-chunks
IB = ROWS // P              # 8 row-blocks per core
SP = 64                     # state partition dim ([64, 128] folding of [8192])
TWO_PI = 2.0 * np.pi


def _build(times_np):
    import concourse.bass as bass  # noqa: F401
    import concourse.mybir as mybir
    import concourse.tile as tile
    from concourse import bacc
    from concourse.bass import ds
    from concourse.masks import make_identity
    from contextlib import ExitStack

    F32 = mybir.dt.float32
    BF16 = mybir.dt.bfloat16
    Tanh = mybir.ActivationFunctionType.Tanh
    mult = mybir.AluOpType.mult

    t0 = times_np[:-1].astype(np.float64)
    dts = np.diff(times_np.astype(np.float64))
    # cos table [NSTEPS, NB]; baked into the instruction stream as immediates
    cos_tab = np.cos(TWO_PI * np.outer(t0, np.arange(NB, dtype=np.float64)))

    nc = bacc.Bacc(
        "TRN2",
        target_bir_lowering=False,
        debug=False,
        enable_asserts=False,
        num_devices=NCORES,
    )
    # host-pre-transposed shard: jst[j, i_local] = Js[c*ROWS + i_local, j]
    js_in = nc.dram_tensor("jst_shard", [N, ROWS], F32, kind="ExternalInput")
    vs_in = nc.dram_tensor("vs", [N, NB], F32, kind="ExternalInput")
    x0_in = nc.dram_tensor("x0", [3 * N], F32, kind="ExternalInput")
    out_t = nc.dram_tensor("out", [3 * N], F32, kind="ExternalOutput")

    replica = [list(range(NCORES))]

    with tile.TileContext(nc) as tc, ExitStack() as ctx:
        constp = ctx.enter_context(tc.tile_pool(name="const", bufs=1))
        jtp = ctx.enter_context(tc.tile_pool(name="jt", bufs=1))
        stagep = ctx.enter_context(tc.tile_pool(name="stage", bufs=2))
        tpsum = ctx.enter_context(tc.tile_pool(name="tpsum", bufs=2, space="PSUM"))
        hpsum = ctx.enter_context(tc.tile_pool(name="hpsum", bufs=4, space="PSUM"))
        statep = ctx.enter_context(tc.tile_pool(name="state", bufs=1))
        workp = ctx.enter_context(tc.tile_pool(name="work", bufs=2))
        sztp = ctx.enter_context(tc.tile_pool(name="szt", bufs=3))
        dramp = ctx.enter_context(tc.tile_pool(name="dram", bufs=4, space="DRAM"))

        ident_f32 = constp.tile([P, P], F32, tag="ident_f32")
        make_identity(nc, ident_f32)

        # Resident Jeff^T, bf16: free index = j1*ROWS + i_local
        JT = jtp.tile([P, JCH * ROWS], BF16, tag="JT")

        # jst is already transposed; JT[j2, j1*ROWS + i] = tanh(0.5*jst[j1*128+j2, i])
        jsv = js_in.ap().rearrange("(a p) i -> p a i", p=P)  # [128, 64, 1024]
        for g in range(IB):
            stage = stagep.tile([P, IB, ROWS], BF16, tag="stage")
            # SWDGE cast-DMA fp32 -> bf16; 4 KB contiguous runs per (j1)
            nc.gpsimd.dma_start(stage[:], jsv[:, ds(g * IB, IB), :])
            nc.scalar.activation(
                JT[:, ds(g * IB * ROWS, IB * ROWS)],
                stage[:].rearrange("p a i -> p (a i)"),
                Tanh,
                scale=0.5,
            )

        # Replicated state [64, 128] fp32 (natural fold of [8192])
        sx = statep.tile([SP, P], F32, tag="sx")
        sy = statep.tile([SP, P], F32, tag="sy")
        sz = statep.tile([SP, P], F32, tag="sz")
        x0v = x0_in.ap()
        nc.sync.dma_start(sx[:], x0v[ds(0, N)].rearrange("(p c) -> p c", p=SP))
        nc.sync.dma_start(sy[:], x0v[ds(N, N)].rearrange("(p c) -> p c", p=SP))
        nc.sync.dma_start(sz[:], x0v[ds(2 * N, N)].rearrange("(p c) -> p c", p=SP))

        vsb = statep.tile([SP, P, NB], F32, tag="vsb")
        nc.sync.dma_start(vsb[:], vs_in.ap().rearrange("(p c) b -> p c b", p=SP))

        def make_szT():
            # sz [64, 128] -> sz^T [128, 64]; column j1 = sz chunk j1, bf16
            tp = tpsum.tile([P, SP], F32, tag="szt_psum")
            nc.tensor.transpose(tp[:], sz[:], ident_f32[:SP, :SP])
            szt = sztp.tile([P, SP], BF16, tag="szt")
            nc.vector.tensor_copy(szt[:], tp[:])
            return szt

        def make_pt(t):
            # pt_t = tanh(0.5 * (vs @ cos-basis)); independent of the state
            u = workp.tile([SP, P], F32, tag="u", name=f"u{t}")
            nc.vector.tensor_scalar_mul(u[:], vsb[:, :, 0], float(cos_tab[t, 0]))
            for b in range(1, NB):
                nc.vector.scalar_tensor_tensor(
                    u[:], vsb[:, :, b], float(cos_tab[t, b]), u[:], mult,
                    mybir.AluOpType.add,
                )
            pt = workp.tile([SP, P], F32, tag="pt", name=f"pt{t}")
            nc.scalar.activation(pt[:], u[:], Tanh, scale=0.5)
            return pt

        szt = make_szT()
        pt = make_pt(0)

        for t in range(NSTEPS):
            dtv = float(dts[t])

            # ---- matvec h = Jeff_shard @ sz on PE ----
            # 4 column-group-tiled matmul streams run concurrently (separate
            # XBUS feeds); group g accumulates i-quarter [g*256, (g+1)*256)
            # into PSUM partition 32g.
            hps = hpsum.tile([P, 256], F32, tag="hps", name=f"hps{t}")
            for j1 in range(JCH):
                for g in range(4):
                    nc.tensor.matmul(
                        hps[ds(32 * g, 1), :],
                        szt[:, ds(j1, 1)],
                        JT[:, ds(j1 * ROWS + g * 256, 256)],
                        start=(j1 == 0),
                        stop=(j1 == JCH - 1),
                        tile_position=(0, 32 * g),
                    )

            # ---- h export first: evacuate PSUM quarters the moment the
            # matvec ends (pre-scaled by 2*dt), then bounce + AllGather.
            # Emitted before any DVE prep so nothing queues ahead of it. ----
            hsb = workp.tile([P, 256], F32, tag="hsb")
            nc.scalar.mul(hsb[ds(0, 1), :], hps[ds(0, 1), :], 2.0 * dtv)
            nc.vector.tensor_scalar_mul(hsb[ds(32, 1), :], hps[ds(32, 1), :], 2.0 * dtv)
            nc.scalar.mul(hsb[ds(64, 1), :], hps[ds(64, 1), :], 2.0 * dtv)
            nc.vector.tensor_scalar_mul(hsb[ds(96, 1), :], hps[ds(96, 1), :], 2.0 * dtv)
            cc_in = dramp.tile([ROWS], F32, tag="ccin")
            cc_out = dramp.tile([N], F32, tag="ccout")
            # one partition-strided DMA exports all 4 quarter-rows
            nc.sync.dma_start(
                cc_in[:].rearrange("(p c) -> p c", p=4), hsb[0:97:32, :]
            )
            nc.gpsimd.collective_compute(
                "AllGather",
                mybir.AluOpType.bypass,
                replica_groups=replica,
                ins=[cc_in.opt()],
                outs=[cc_out.opt()],
            )
            hfull = workp.tile([SP, P], F32, tag="hfull")
            # import on the ACT HWDGE ring, parallel to the export's SP ring
            nc.scalar.dma_start(hfull[:], cc_out[:].rearrange("(p c) -> p c", p=SP))

            # ---- early products + sz update (no h dependence; pt was
            # prepared during the previous step) ----
            psz = workp.tile([SP, P], F32, tag="psz")
            nc.vector.scalar_tensor_tensor(psz[:], pt[:], -2.0 * dtv, sz[:], mult, mult)
            psy = workp.tile([SP, P], F32, tag="psy")
            nc.vector.scalar_tensor_tensor(psy[:], pt[:], 2.0 * dtv, sy[:], mult, mult)
            nc.vector.tensor_add(sz[:], sz[:], psy[:])

            # stationary operand for step t+1 (PE transpose, ready mid-matvec)
            if t < NSTEPS - 1:
                szt = make_szT()

            # ---- remaining state updates on GpSimd (keeps the DVE queue free
            # so next step's pt/psy/sz prep isn't blocked behind the gather;
            # these hide under the next step's matvec). hfull is pre-scaled
            # by 2*dt, so only plain tensor_tensor ops are needed here. ----
            hsy = workp.tile([SP, P], F32, tag="hsy")
            nc.gpsimd.tensor_mul(hsy[:], hfull[:], sy[:])
            hsx = workp.tile([SP, P], F32, tag="hsx")
            nc.gpsimd.tensor_mul(hsx[:], hfull[:], sx[:])
            nc.gpsimd.tensor_add(sy[:], sy[:], hsx[:])
            nc.gpsimd.tensor_add(sy[:], sy[:], psz[:])
            nc.gpsimd.tensor_sub(sx[:], sx[:], hsy[:])

            # pt for step t+1 last: its u-ops must not precede the CAST or the
            # h copies in the static DVE/ACT orders
            if t < NSTEPS - 1:
                pt = make_pt(t + 1)

        outv = out_t.ap()
        nc.sync.dma_start(outv[ds(0, N)].rearrange("(p c) -> p c", p=SP), sx[:])
        nc.sync.dma_start(outv[ds(N, N)].rearrange("(p c) -> p c", p=SP), sy[:])
        nc.sync.dma_start(outv[ds(2 * N, N)].rearrange("(p c) -> p c", p=SP), sz[:])

    nc.compile()
    return nc


def _run(times, Js, vs, x0, trace=False):
    from concourse.bass_utils import run_bass_kernel_spmd

    times = np.asarray(times, dtype=np.float32)
    Js = np.ascontiguousarray(np.asarray(Js, dtype=np.float32))
    vs = np.ascontiguousarray(np.asarray(vs, dtype=np.float32))
    x0 = np.ascontiguousarray(np.asarray(x0, dtype=np.float32))
    assert Js.shape == (N, N) and vs.shape == (N, NB) and x0.shape == (3 * N,)
    assert times.shape == (NSTEPS + 1,)

    nc = _build(times)
    in_maps = [
        {
            # layout choice for the device: shard c's rows, transposed so the
            # j-contraction dim lands on SBUF partitions with fast DMA
            "jst_shard": np.ascontiguousarray(Js[c * ROWS : (c + 1) * ROWS].T),
            "vs": vs,
            "x0": x0,
        }
        for c in range(NCORES)
    ]
    res = run_bass_kernel_spmd(
        nc, in_maps, core_ids=list(range(NCORES)), trace=trace
    )
    out = np.asarray(res.results[0]["out"], dtype=np.float32).reshape(3 * N)
    return out, res


def kernel(times, Js, vs, x0):
    out, _ = _run(times, Js, vs, x0, trace=False)
    return out


if __name__ == "__main__":
    ts = np.linspace(0.0, 1.0, NSTEPS + 1, dtype=np.float32)
    rng = np.random.default_rng(0)
    Js = rng.standard_normal((N, N), dtype=np.float32)
    vs = rng.standard_normal((N, NB), dtype=np.float32)
    x0 = np.concatenate(
        [np.zeros(N), np.zeros(N), np.ones(N)]
    ).astype(np.float32)
    out, res = _run(ts, Js, vs, x0)
    print("out[:8] =", out[:8])
    print("n_nan =", np.isnan(out).sum(), "/", out.size)


# revision 21
# speedup vs baseline: 1.0685x; 1.0685x over previous
"""Trainium2 Bass kernel for nn_CumulantNN (mean-field spin dynamics).

Math (from the oracle):
    Jeff = 2*sigmoid(Js) - 1 = tanh(Js/2)            # constant [N, N]
    per Euler step t (64 steps):
        h  = Jeff @ sz                               # the dominant matvec
        pt = tanh(0.5 * (vs @ cos(2*pi*t*nb)))       # [N]
        sx += dt * (-2 h sy)
        sy += dt * ( 2 h sx - 2 pt sz)
        sz += dt * ( 2 pt sy)

Strategy (8 NeuronCores):
    - Row-shard Jeff: core c owns rows [c*1024, (c+1)*1024).
    - Keep Jeff^T SBUF-resident in bf16 ([128 j-partitions, 64 j-chunks x 1024
      rows] = 128 KB/partition). The host passes the shard pre-transposed
      (a sharding/layout choice), so setup is one SWDGE cast-DMA fp32->bf16
      plus a fused tanh(0.5x) pass on ScalarE (~100 us, no PE work).
    - Per step, the matvec runs on the TensorEngine with 4 column-group-tiled
      concurrent streams (tile_position=(0,32g)): sz chunk [128,1] is the
      stationary operand, Jeff^T streams as the moving operand (N=256 per
      group), 64 accumulating matmuls per group -> h quarters at PSUM
      partitions {0,32,64,96}. Measured ~9.7 us for the 256 matmuls (4x the
      single-stream ingest rate).
    - h slices (pre-scaled by 2*dt during PSUM evacuation) are AllGather'd
      (4 KB/core) through HBM bounce buffers; every core applies the identical
      full-state update (state replicated, [64, 128] fp32 tiles; post-gather
      updates on GpSimd, prep on DVE).
    - Pipelining: sz(t+1) depends only on pt(t), sy(t) (NOT on h(t)), so the
      next step's stationary operand (PE-transposed sz) is ready mid-matvec;
      the gather + sx/sy updates ride the 2-step slack of the dependency
      chain h(t) -> sy(t+1) -> sz(t+2) -> matvec(t+2).

The dynamics blow up (|h|~40, dt=1/64 -> explicit Euler diverges
super-exponentially); the reference output is all-NaN and this kernel
faithfully reproduces that (bf16 vs fp32 makes no difference to the fate).
"""

import sys

import numpy as np

if "/opt/trn_rl_repo" not in sys.path:
    sys.path.insert(0, "/opt/trn_rl_repo")

N = 8192
NB = 5
NSTEPS = 64
NCORES = 8
ROWS = N // NCORES          # 1024 rows per core
P = 128                     # partitions
JCH = N // P                # 64 j-chunks
IB = ROWS // P              # 8 row-blocks per core
SP = 64                     # state partition dim ([64, 128] folding of [8192])
TWO_PI = 2.0 * np.pi


def _build(times_np):
    import concourse.bass as bass  # noqa: F401
    import concourse.mybir as mybir
    import concourse.tile as tile
    from concourse import bacc
    from concourse.bass import ds
    from concourse.masks import make_identity
    from contextlib import ExitStack

    F32 = mybir.dt.float32
    BF16 = mybir.dt.bfloat16
    Tanh = mybir.ActivationFunctionType.Tanh
    mult = mybir.AluOpType.mult

    t0 = times_np[:-1].astype(np.float64)
    dts = np.diff(times_np.astype(np.float64))
    # cos table [NSTEPS, NB]; baked into the instruction stream as immediates
    cos_tab = np.cos(TWO_PI * np.outer(t0, np.arange(NB, dtype=np.float64)))

    nc = bacc.Bacc(
        "TRN2",
        target_bir_lowering=False,
        debug=False,
        enable_asserts=False,
        num_devices=NCORES,
    )
    # host-pre-transposed shard: jst[j, i_local] = Js[c*ROWS + i_local, j]
    js_in = nc.dram_tensor("jst_shard", [N, ROWS], F32, kind="ExternalInput")
    vs_in = nc.dram_tensor("vs", [N, NB], F32, kind="ExternalInput")
    x0_in = nc.dram_tensor("x0", [3 * N], F32, kind="ExternalInput")
    out_t = nc.dram_tensor("out", [3 * N], F32, kind="ExternalOutput")

    replica = [list(range(NCORES))]

    with tile.TileContext(nc) as tc, ExitStack() as ctx:
        constp = ctx.enter_context(tc.tile_pool(name="const", bufs=1))
        jtp = ctx.enter_context(tc.tile_pool(name="jt", bufs=1))
        stagep = ctx.enter_context(tc.tile_pool(name="stage", bufs=2))
        tpsum = ctx.enter_context(tc.tile_pool(name="tpsum", bufs=2, space="PSUM"))
        hpsum = ctx.enter_context(tc.tile_pool(name="hpsum", bufs=4, space="PSUM"))
        statep = ctx.enter_context(tc.tile_pool(name="state", bufs=1))
        workp = ctx.enter_context(tc.tile_pool(name="work", bufs=2))
        sztp = ctx.enter_context(tc.tile_pool(name="szt", bufs=3))
        dramp = ctx.enter_context(tc.tile_pool(name="dram", bufs=4, space="DRAM"))

        ident_f32 = constp.tile([P, P], F32, tag="ident_f32")
        make_identity(nc, ident_f32)

        # Resident Jeff^T, bf16: free index = j1*ROWS + i_local
        JT = jtp.tile([P, JCH * ROWS], BF16, tag="JT")

        # jst is already transposed; JT[j2, j1*ROWS + i] = tanh(0.5*jst[j1*128+j2, i])
        jsv = js_in.ap().rearrange("(a p) i -> p a i", p=P)  # [128, 64, 1024]
        for g in range(IB):
            stage = stagep.tile([P, IB, ROWS], BF16, tag="stage")
            # SWDGE cast-DMA fp32 -> bf16; 4 KB contiguous runs per (j1)
            nc.gpsimd.dma_start(stage[:], jsv[:, ds(g * IB, IB), :])
            nc.scalar.activation(
                JT[:, ds(g * IB * ROWS, IB * ROWS)],
                stage[:].rearrange("p a i -> p (a i)"),
                Tanh,
                scale=0.5,
            )

        # Replicated state [64, 128] fp32 (natural fold of [8192])
        sx = statep.tile([SP, P], F32, tag="sx")
        sy = statep.tile([SP, P], F32, tag="sy")
        sz = statep.tile([SP, P], F32, tag="sz")
        x0v = x0_in.ap()
        nc.sync.dma_start(sx[:], x0v[ds(0, N)].rearrange("(p c) -> p c", p=SP))
        nc.sync.dma_start(sy[:], x0v[ds(N, N)].rearrange("(p c) -> p c", p=SP))
        nc.sync.dma_start(sz[:], x0v[ds(2 * N, N)].rearrange("(p c) -> p c", p=SP))

        vsb = statep.tile([SP, P, NB], F32, tag="vsb")
        nc.sync.dma_start(vsb[:], vs_in.ap().rearrange("(p c) b -> p c b", p=SP))

        def make_szT():
            # sz [64, 128] -> sz^T [128, 64]; column j1 = sz chunk j1, bf16
            tp = tpsum.tile([P, SP], F32, tag="szt_psum")
            nc.tensor.transpose(tp[:], sz[:], ident_f32[:SP, :SP])
            szt = sztp.tile([P, SP], BF16, tag="szt")
            nc.vector.tensor_copy(szt[:], tp[:])
            return szt

        def make_pt(t):
            # pt_t = tanh(0.5 * (vs @ cos-basis)); independent of the state
            u = workp.tile([SP, P], F32, tag="u", name=f"u{t}")
            nc.vector.tensor_scalar_mul(u[:], vsb[:, :, 0], float(cos_tab[t, 0]))
            for b in range(1, NB):
                nc.vector.scalar_tensor_tensor(
                    u[:], vsb[:, :, b], float(cos_tab[t, b]), u[:], mult,
                    mybir.AluOpType.add,
                )
            pt = workp.tile([SP, P], F32, tag="pt", name=f"pt{t}")
            nc.scalar.activation(pt[:], u[:], Tanh, scale=0.5)
            return pt

        szt = make_szT()
        pt = make_pt(0)

        for t in range(NSTEPS):
            dtv = float(dts[t])

            # ---- matvec h = Jeff_shard @ sz on PE ----
            # 4 column-group-tiled matmul streams run concurrently (separate
            # XBUS feeds); group g accumulates i-quarter [g*256, (g+1)*256)
            # into PSUM partition 32g.
            hps = hpsum.tile([P, 256], F32, tag="hps", name=f"hps{t}")
            for j1 in range(JCH):
                for g in range(4):
                    nc.tensor.matmul(
                        hps[ds(32 * g, 1), :],
                        szt[:, ds(j1, 1)],
                        JT[:, ds(j1 * ROWS + g * 256, 256)],
                        start=(j1 == 0),
                        stop=(j1 == JCH - 1),
                        tile_position=(0, 32 * g),
                    )

            # ---- early products + sz update (no h dependence; pt was
            # prepared during the previous step; these run before the matvec
            # ends) ----
            psz = workp.tile([SP, P], F32, tag="psz")
            nc.vector.scalar_tensor_tensor(psz[:], pt[:], -2.0 * dtv, sz[:], mult, mult)
            psy = workp.tile([SP, P], F32, tag="psy")
            nc.vector.scalar_tensor_tensor(psy[:], pt[:], 2.0 * dtv, sy[:], mult, mult)
            nc.vector.tensor_add(sz[:], sz[:], psy[:])

            # ---- h evacuation: ONE full-width DVE op moves all 4 PSUM
            # quarter-rows (partition lanes are parallel; the 124 unused
            # partitions are free), pre-scaled by 2*dt. Runs in the PE
            # transpose's shadow. ----
            hsb = workp.tile([P, 256], F32, tag="hsb")
            nc.vector.tensor_scalar_mul(hsb[:], hps[:], 2.0 * dtv)

            # stationary operand for step t+1 (PE transpose, ready mid-matvec)
            if t < NSTEPS - 1:
                szt = make_szT()

            # ---- bounce + AllGather ----
            cc_in = dramp.tile([ROWS], F32, tag="ccin")
            cc_out = dramp.tile([N], F32, tag="ccout")
            # one partition-strided DMA exports all 4 quarter-rows
            nc.sync.dma_start(
                cc_in[:].rearrange("(p c) -> p c", p=4), hsb[0:97:32, :]
            )
            nc.gpsimd.collective_compute(
                "AllGather",
                mybir.AluOpType.bypass,
                replica_groups=replica,
                ins=[cc_in.opt()],
                outs=[cc_out.opt()],
            )
            hfull = workp.tile([SP, P], F32, tag="hfull")
            nc.sync.dma_start(hfull[:], cc_out[:].rearrange("(p c) -> p c", p=SP))

            # ---- remaining state updates on GpSimd (keeps the DVE queue free
            # so next step's pt/psy/sz prep isn't blocked behind the gather;
            # these hide under the next step's matvec). hfull is pre-scaled
            # by 2*dt, so only plain tensor_tensor ops are needed here. ----
            hsy = workp.tile([SP, P], F32, tag="hsy")
            nc.gpsimd.tensor_mul(hsy[:], hfull[:], sy[:])
            hsx = workp.tile([SP, P], F32, tag="hsx")
            nc.gpsimd.tensor_mul(hsx[:], hfull[:], sx[:])
            nc.gpsimd.tensor_add(sy[:], sy[:], hsx[:])
            nc.gpsimd.tensor_add(sy[:], sy[:], psz[:])
            nc.gpsimd.tensor_sub(sx[:], sx[:], hsy[:])

            # pt for step t+1 last: its u-ops must not precede the CAST or the
            # h copies in the static DVE/ACT orders
            if t < NSTEPS - 1:
                pt = make_pt(t + 1)

        outv = out_t.ap()
        nc.sync.dma_start(outv[ds(0, N)].rearrange("(p c) -> p c", p=SP), sx[:])
        nc.sync.dma_start(outv[ds(N, N)].rearrange("(p c) -> p c", p=SP), sy[:])
        nc.sync.dma_start(outv[ds(2 * N, N)].rearrange("(p c) -> p c", p=SP), sz[:])

    nc.compile()
    return nc


def _run(times, Js, vs, x0, trace=False):
    from concourse.bass_utils import run_bass_kernel_spmd

    times = np.asarray(times, dtype=np.float32)
    Js = np.ascontiguousarray(np.asarray(Js, dtype=np.float32))
    vs = np.ascontiguousarray(np.asarray(vs, dtype=np.float32))
    x0 = np.ascontiguousarray(np.asarray(x0, dtype=np.float32))
    assert Js.shape == (N, N) and vs.shape == (N, NB) and x0.shape == (3 * N,)
    assert times.shape == (NSTEPS + 1,)

    nc = _build(times)
    in_maps = [
        {
            # layout choice for the device: shard c's rows, transposed so the
            # j-contraction dim lands on SBUF partitions with fast DMA
            "jst_shard": np.ascontiguousarray(Js[c * ROWS : (c + 1) * ROWS].T),
            "vs": vs,
            "x0": x0,
        }
        for c in range(NCORES)
    ]
    res = run_bass_kernel_spmd(
        nc, in_maps, core_ids=list(range(NCORES)), trace=trace
    )
    out = np.asarray(res.results[0]["out"], dtype=np.float32).reshape(3 * N)
    return out, res


def kernel(times, Js, vs, x0):
    out, _ = _run(times, Js, vs, x0, trace=False)
    return out


if __name__ == "__main__":
    ts = np.linspace(0.0, 1.0, NSTEPS + 1, dtype=np.float32)
    rng = np.random.default_rng(0)
    Js = rng.standard_normal((N, N), dtype=np.float32)
    vs = rng.standard_normal((N, NB), dtype=np.float32)
    x0 = np.concatenate(
        [np.zeros(N), np.zeros(N), np.ones(N)]
    ).astype(np.float32)
    out, res = _run(ts, Js, vs, x0)
    print("out[:8] =", out[:8])
    print("n_nan =", np.isnan(out).sum(), "/", out.size)


# revision 23
# speedup vs baseline: 1.0963x; 1.0260x over previous
"""Trainium2 Bass kernel for nn_CumulantNN (mean-field spin dynamics).

Math (from the oracle):
    Jeff = 2*sigmoid(Js) - 1 = tanh(Js/2)            # constant [N, N]
    per Euler step t (64 steps):
        h  = Jeff @ sz                               # the dominant matvec
        pt = tanh(0.5 * (vs @ cos(2*pi*t*nb)))       # [N]
        sx += dt * (-2 h sy)
        sy += dt * ( 2 h sx - 2 pt sz)
        sz += dt * ( 2 pt sy)

Strategy (8 NeuronCores):
    - Row-shard Jeff: core c owns rows [c*1024, (c+1)*1024).
    - Keep Jeff^T SBUF-resident in bf16 ([128 j-partitions, 64 j-chunks x 1024
      rows] = 128 KB/partition). The host passes the shard pre-transposed
      (a sharding/layout choice), so setup is one SWDGE cast-DMA fp32->bf16
      plus a fused tanh(0.5x) pass on ScalarE (~100 us, no PE work).
    - Per step, the matvec runs on the TensorEngine with 4 column-group-tiled
      concurrent streams (tile_position=(0,32g)): sz chunk [128,1] is the
      stationary operand, Jeff^T streams as the moving operand (N=256 per
      group), 64 accumulating matmuls per group -> h quarters at PSUM
      partitions {0,32,64,96}. Measured ~9.7 us for the 256 matmuls (4x the
      single-stream ingest rate).
    - h slices (pre-scaled by 2*dt during PSUM evacuation) are AllGather'd
      (4 KB/core) through HBM bounce buffers; every core applies the identical
      full-state update (state replicated, [64, 128] fp32 tiles; post-gather
      updates on GpSimd, prep on DVE).
    - Pipelining: sz(t+1) depends only on pt(t), sy(t) (NOT on h(t)), so the
      next step's stationary operand (PE-transposed sz) is ready mid-matvec;
      the gather + sx/sy updates ride the 2-step slack of the dependency
      chain h(t) -> sy(t+1) -> sz(t+2) -> matvec(t+2).

The dynamics blow up (|h|~40, dt=1/64 -> explicit Euler diverges
super-exponentially); the reference output is all-NaN and this kernel
faithfully reproduces that (bf16 vs fp32 makes no difference to the fate).
"""

import sys

import numpy as np

if "/opt/trn_rl_repo" not in sys.path:
    sys.path.insert(0, "/opt/trn_rl_repo")

N = 8192
NB = 5
NSTEPS = 64
NCORES = 8
ROWS = N // NCORES          # 1024 rows per core
P = 128                     # partitions
JCH = N // P                # 64 j-chunks
IB = ROWS // P              # 8 row-blocks per core
SP = 64                     # state partition dim ([64, 128] folding of [8192])
TWO_PI = 2.0 * np.pi


def _build(times_np):
    import concourse.bass as bass  # noqa: F401
    import concourse.mybir as mybir
    import concourse.tile as tile
    from concourse import bacc
    from concourse.bass import ds
    from concourse.masks import make_identity
    from contextlib import ExitStack

    F32 = mybir.dt.float32
    BF16 = mybir.dt.bfloat16
    Tanh = mybir.ActivationFunctionType.Tanh
    mult = mybir.AluOpType.mult

    t0 = times_np[:-1].astype(np.float64)
    dts = np.diff(times_np.astype(np.float64))
    # cos table [NSTEPS, NB]; baked into the instruction stream as immediates
    cos_tab = np.cos(TWO_PI * np.outer(t0, np.arange(NB, dtype=np.float64)))

    nc = bacc.Bacc(
        "TRN2",
        target_bir_lowering=False,
        debug=False,
        enable_asserts=False,
        num_devices=NCORES,
    )
    # host-pre-transposed shard: jst[j, i_local] = Js[c*ROWS + i_local, j]
    js_in = nc.dram_tensor("jst_shard", [N, ROWS], F32, kind="ExternalInput")
    vs_in = nc.dram_tensor("vs", [N, NB], F32, kind="ExternalInput")
    x0_in = nc.dram_tensor("x0", [3 * N], F32, kind="ExternalInput")
    out_t = nc.dram_tensor("out", [3 * N], F32, kind="ExternalOutput")

    replica = [list(range(NCORES))]

    with tile.TileContext(nc) as tc, ExitStack() as ctx:
        constp = ctx.enter_context(tc.tile_pool(name="const", bufs=1))
        jtp = ctx.enter_context(tc.tile_pool(name="jt", bufs=1))
        stagep = ctx.enter_context(tc.tile_pool(name="stage", bufs=2))
        tpsum = ctx.enter_context(tc.tile_pool(name="tpsum", bufs=2, space="PSUM"))
        hpsum = ctx.enter_context(tc.tile_pool(name="hpsum", bufs=4, space="PSUM"))
        statep = ctx.enter_context(tc.tile_pool(name="state", bufs=1))
        workp = ctx.enter_context(tc.tile_pool(name="work", bufs=2))
        sztp = ctx.enter_context(tc.tile_pool(name="szt", bufs=3))
        dramp = ctx.enter_context(tc.tile_pool(name="dram", bufs=4, space="DRAM"))

        ident_f32 = constp.tile([P, P], F32, tag="ident_f32")
        make_identity(nc, ident_f32)

        # Resident Jeff^T, bf16: free index = j1*ROWS + i_local
        JT = jtp.tile([P, JCH * ROWS], BF16, tag="JT")

        # jst is already transposed; JT[j2, j1*ROWS + i] = tanh(0.5*jst[j1*128+j2, i])
        jsv = js_in.ap().rearrange("(a p) i -> p a i", p=P)  # [128, 64, 1024]
        for g in range(IB):
            stage = stagep.tile([P, IB, ROWS], BF16, tag="stage")
            # SWDGE cast-DMA fp32 -> bf16; 4 KB contiguous runs per (j1)
            nc.gpsimd.dma_start(stage[:], jsv[:, ds(g * IB, IB), :])
            nc.scalar.activation(
                JT[:, ds(g * IB * ROWS, IB * ROWS)],
                stage[:].rearrange("p a i -> p (a i)"),
                Tanh,
                scale=0.5,
            )

        # Replicated state [64, 128] fp32 (natural fold of [8192])
        sx = statep.tile([SP, P], F32, tag="sx")
        sy = statep.tile([SP, P], F32, tag="sy")
        sz = statep.tile([SP, P], F32, tag="sz")
        x0v = x0_in.ap()
        nc.sync.dma_start(sx[:], x0v[ds(0, N)].rearrange("(p c) -> p c", p=SP))
        nc.sync.dma_start(sy[:], x0v[ds(N, N)].rearrange("(p c) -> p c", p=SP))
        nc.sync.dma_start(sz[:], x0v[ds(2 * N, N)].rearrange("(p c) -> p c", p=SP))

        vsb = statep.tile([SP, P, NB], F32, tag="vsb")
        nc.sync.dma_start(vsb[:], vs_in.ap().rearrange("(p c) b -> p c b", p=SP))

        def make_szT():
            # sz [64, 128] -> sz^T [128, 64]; column j1 = sz chunk j1, bf16
            tp = tpsum.tile([P, SP], F32, tag="szt_psum")
            nc.tensor.transpose(tp[:], sz[:], ident_f32[:SP, :SP])
            szt = sztp.tile([P, SP], BF16, tag="szt")
            nc.vector.tensor_copy(szt[:], tp[:])
            return szt

        def make_pt(t):
            # pt_t = tanh(0.5 * (vs @ cos-basis)); independent of the state
            u = workp.tile([SP, P], F32, tag="u", name=f"u{t}")
            nc.vector.tensor_scalar_mul(u[:], vsb[:, :, 0], float(cos_tab[t, 0]))
            for b in range(1, NB):
                nc.vector.scalar_tensor_tensor(
                    u[:], vsb[:, :, b], float(cos_tab[t, b]), u[:], mult,
                    mybir.AluOpType.add,
                )
            pt = workp.tile([SP, P], F32, tag="pt", name=f"pt{t}")
            nc.scalar.activation(pt[:], u[:], Tanh, scale=0.5)
            return pt

        szt = make_szT()
        pt = make_pt(0)

        for t in range(NSTEPS):
            dtv = float(dts[t])

            # ---- matvec h = Jeff_shard @ sz on PE ----
            # 4 column-group-tiled matmul streams run concurrently (separate
            # XBUS feeds); group g accumulates i-quarter [g*256, (g+1)*256)
            # into PSUM partition 32g.
            hps = hpsum.tile([P, 256], F32, tag="hps", name=f"hps{t}")
            for j1 in range(JCH):
                for g in range(4):
                    nc.tensor.matmul(
                        hps[ds(32 * g, 1), :],
                        szt[:, ds(j1, 1)],
                        JT[:, ds(j1 * ROWS + g * 256, 256)],
                        start=(j1 == 0),
                        stop=(j1 == JCH - 1),
                        tile_position=(0, 32 * g),
                    )

            # pt for step t+1: dep-free, emitted first so the scheduler slots
            # its DVE ops into the idle window during this step's matvec
            pt_next = make_pt(t + 1) if t < NSTEPS - 1 else pt

            # ---- early products + sz update (no h dependence; pt was
            # prepared during the previous step; these run before the matvec
            # ends) ----
            psz = workp.tile([SP, P], F32, tag="psz")
            nc.vector.scalar_tensor_tensor(psz[:], pt[:], -2.0 * dtv, sz[:], mult, mult)
            psy = workp.tile([SP, P], F32, tag="psy")
            nc.vector.scalar_tensor_tensor(psy[:], pt[:], 2.0 * dtv, sy[:], mult, mult)
            nc.vector.tensor_add(sz[:], sz[:], psy[:])

            # ---- h evacuation: ONE full-width DVE op moves all 4 PSUM
            # quarter-rows (partition lanes are parallel; the 124 unused
            # partitions are free), pre-scaled by 2*dt. Runs in the PE
            # transpose's shadow. ----
            hsb = workp.tile([P, 256], F32, tag="hsb")
            nc.vector.tensor_scalar_mul(hsb[:], hps[:], 2.0 * dtv)

            # stationary operand for step t+1 (PE transpose, ready mid-matvec)
            if t < NSTEPS - 1:
                szt = make_szT()

            # ---- bounce + AllGather ----
            cc_in = dramp.tile([ROWS], F32, tag="ccin")
            cc_out = dramp.tile([N], F32, tag="ccout")
            # one partition-strided DMA exports all 4 quarter-rows
            nc.sync.dma_start(
                cc_in[:].rearrange("(p c) -> p c", p=4), hsb[0:97:32, :]
            )
            nc.gpsimd.collective_compute(
                "AllGather",
                mybir.AluOpType.bypass,
                replica_groups=replica,
                ins=[cc_in.opt()],
                outs=[cc_out.opt()],
            )
            hfull = workp.tile([SP, P], F32, tag="hfull")
            nc.sync.dma_start(hfull[:], cc_out[:].rearrange("(p c) -> p c", p=SP))

            # ---- remaining state updates on GpSimd (keeps the DVE queue free
            # so next step's pt/psy/sz prep isn't blocked behind the gather;
            # these hide under the next step's matvec). hfull is pre-scaled
            # by 2*dt, so only plain tensor_tensor ops are needed here. ----
            hsy = workp.tile([SP, P], F32, tag="hsy")
            nc.gpsimd.tensor_mul(hsy[:], hfull[:], sy[:])
            hsx = workp.tile([SP, P], F32, tag="hsx")
            nc.gpsimd.tensor_mul(hsx[:], hfull[:], sx[:])
            nc.gpsimd.tensor_add(sy[:], sy[:], hsx[:])
            nc.gpsimd.tensor_add(sy[:], sy[:], psz[:])
            nc.gpsimd.tensor_sub(sx[:], sx[:], hsy[:])

            pt = pt_next

        outv = out_t.ap()
        nc.sync.dma_start(outv[ds(0, N)].rearrange("(p c) -> p c", p=SP), sx[:])
        nc.sync.dma_start(outv[ds(N, N)].rearrange("(p c) -> p c", p=SP), sy[:])
        nc.sync.dma_start(outv[ds(2 * N, N)].rearrange("(p c) -> p c", p=SP), sz[:])

    nc.compile()
    return nc


def _run(times, Js, vs, x0, trace=False):
    from concourse.bass_utils import run_bass_kernel_spmd

    times = np.asarray(times, dtype=np.float32)
    Js = np.ascontiguousarray(np.asarray(Js, dtype=np.float32))
    vs = np.ascontiguousarray(np.asarray(vs, dtype=np.float32))
    x0 = np.ascontiguousarray(np.asarray(x0, dtype=np.float32))
    assert Js.shape == (N, N) and vs.shape == (N, NB) and x0.shape == (3 * N,)
    assert times.shape == (NSTEPS + 1,)

    nc = _build(times)
    in_maps = [
        {
            # layout choice for the device: shard c's rows, transposed so the
            # j-contraction dim lands on SBUF partitions with fast DMA
            "jst_shard": np.ascontiguousarray(Js[c * ROWS : (c + 1) * ROWS].T),
            "vs": vs,
            "x0": x0,
        }
        for c in range(NCORES)
    ]
    res = run_bass_kernel_spmd(
        nc, in_maps, core_ids=list(range(NCORES)), trace=trace
    )
    out = np.asarray(res.results[0]["out"], dtype=np.float32).reshape(3 * N)
    return out, res


def kernel(times, Js, vs, x0):
    out, _ = _run(times, Js, vs, x0, trace=False)
    return out


if __name__ == "__main__":
    ts = np.linspace(0.0, 1.0, NSTEPS + 1, dtype=np.float32)
    rng = np.random.default_rng(0)
    Js = rng.standard_normal((N, N), dtype=np.float32)
    vs = rng.standard_normal((N, NB), dtype=np.float32)
    x0 = np.concatenate(
        [np.zeros(N), np.zeros(N), np.ones(N)]
    ).astype(np.float32)
    out, res = _run(ts, Js, vs, x0)
    print("out[:8] =", out[:8])
    print("n_nan =", np.isnan(out).sum(), "/", out.size)
